# revision 1
# baseline (speedup 1.0000x reference)
"""Trainium2 Bass kernel for nn_Attention_1013612281902.

Reference computation (per batch b, head h):
    Q = emb @ Wq[h].T            [S,C]
    K = emb_all @ Wk[h].T        [S,KV]
    V = emb_all @ Wv[h].T        [S,KV]
    scores = Q.T @ K / sqrt(KV)  [C,KV]
    normed = instance_norm(scores)       (mean/var over the whole [C,KV] plane)
    probs  = softmax(normed, axis=KV)
    context = probs @ V.T        [C,S]
    out = mean_h(context).T @ Wo.T       [S,C]

Algebraic restructuring used here (S=4096 >> C=512, KV=960):
    G = emb.T @ emb_all                      [C,KV]   (shared across heads)
    scores = (Wq[h] @ G @ Wk[h].T)/sqrt(KV)
    Pv[h]  = probs[h] @ Wv[h]                [C,KV]
    out    = emb_all @ (mean_h Pv[h]).T @ Wo.T
This avoids materializing Q/K/V ([S,*] tensors) entirely and reduces the
FLOP count ~4x. All matmuls run in float32r (fast fp32 PE mode, ~1.5e-4
relative error vs 4x-slower exact fp32). The BIR verifier requires f32r
matmul operands to be produced as f32r, so every matmul-feeding tile is
allocated float32r; non-matmul reads go through a .bitcast(float32) view.

Sharding: 8 cores = (4 batches) x (2 head-pairs). Core 2b+g computes the
partial output for batch b over heads {2g, 2g+1}; the host adds the two
partials per batch (the head-mean and output projection are linear).
"""

import sys

if "/opt/trn_rl_repo" not in sys.path:
    sys.path.insert(0, "/opt/trn_rl_repo")

import math
from contextlib import ExitStack

import numpy as np

import concourse.bacc as bacc
import concourse.mybir as mybir
import concourse.tile as tile
from concourse.bass_utils import run_bass_kernel_spmd
from concourse.masks import make_identity
from concourse.tile_rust import add_dep_helper

B, S, C, KV, H = 4, 4096, 512, 960, 4
EPS = 1e-5
F32 = mybir.dt.float32
F32R = mybir.dt.float32r

ST = S // 128           # 32 s-tiles
CT = C // 128           # 4 c-tiles
KT = (KV + 127) // 128  # 8 k-tiles (last one has 64 partitions)


def _kp(t):
    return min(128, KV - t * 128)


def _build_program():
    nc = bacc.Bacc("TRN2", target_bir_lowering=False, debug=False, num_devices=8)

    emb_d = nc.dram_tensor("emb", [S, C], F32R, kind="ExternalInput")
    ea_d = nc.dram_tensor("ea", [S, KV], F32R, kind="ExternalInput")
    wqt_d = nc.dram_tensor("wqt", [2, C, C], F32R, kind="ExternalInput")
    wkt_d = nc.dram_tensor("wkt", [2, KV, KV], F32R, kind="ExternalInput")
    wv_d = nc.dram_tensor("wv", [2, KV, KV], F32R, kind="ExternalInput")
    wot_d = nc.dram_tensor("wot", [C, C], F32R, kind="ExternalInput")
    y_d = nc.dram_tensor("y", [S, C], F32, kind="ExternalOutput")

    def f(ap):
        """float32 view of an f32r tile for vector/scalar-engine reads."""
        return ap.bitcast(F32)

    with tile.TileContext(nc) as tc, ExitStack() as ectx:
        ec = ectx.enter_context
        const = ec(tc.tile_pool(name="const", bufs=1))
        gp = ec(tc.tile_pool(name="gp", bufs=1))
        wqp = ec(tc.tile_pool(name="wqp", bufs=1))
        wkp = ec(tc.tile_pool(name="wkp", bufs=1))
        wvp = ec(tc.tile_pool(name="wvp", bufs=1))
        wop = ec(tc.tile_pool(name="wop", bufs=1))
        embp = ec(tc.tile_pool(name="embp", bufs=8))
        eap = ec(tc.tile_pool(name="eap", bufs=5))
        bigp = ec(tc.tile_pool(name="bigp", bufs=1))   # A then Z (sequential reuse)
        scp = ec(tc.tile_pool(name="scp", bufs=1))     # scoresT -> probsT
        pbp = ec(tc.tile_pool(name="pbp", bufs=1))     # Pbar accumulator
        trp = ec(tc.tile_pool(name="trp", bufs=8))
        outp = ec(tc.tile_pool(name="outp", bufs=4))
        srp = ec(tc.tile_pool(name="srp", bufs=2))     # [128,512] scratch
        stp = ec(tc.tile_pool(name="stp", bufs=4))     # small stats tiles

        identf = const.tile([128, 128], F32)
        make_identity(nc, identf[:])
        ident = const.tile([128, 128], F32R)
        nc.vector.tensor_copy(out=ident[:], in_=identf[:])
        onesf = const.tile([128, 128], F32)
        nc.vector.memset(onesf[:], 1.0)
        ones = const.tile([128, 128], F32R)
        nc.vector.tensor_copy(out=ones[:], in_=onesf[:])
        # scores are left unscaled (instance-norm is scale-invariant), so the
        # reference's eps applies to var/KV: use KV*eps against unscaled var.
        eps_t = const.tile([128, 1], F32)
        nc.vector.memset(eps_t[:], EPS * KV)
        # Zero source for padding the 960->1024 partition tails (f32r matmuls
        # at K=64 run at half rate; padding K to 128 keeps full rate).
        zsrc = const.tile([128, KV], F32)
        nc.vector.memset(zsrc[:], 0.0)
        # Scratch for ACT-table prewarming (Sqrt/Exp table loads are ~1.3us;
        # issuing a dummy op early moves the load off the critical chain).
        warm = const.tile([128, 1], F32)
        nc.vector.memset(warm[:], 1.0)

        def prewarm(func, nm):
            wsink = stp.tile([128, 1], F32, tag="wsink", name=nm)
            nc.scalar.activation(out=wsink[:], in_=warm[:], func=func)

        # ---- phase 1: G = emb.T @ emb_all  [C, KV] --------------------------
        g_sb = gp.tile([128, CT, KV], F32R)
        gps_pool = tc.tile_pool(name="gps", bufs=8, space="PSUM")
        ps = gps_pool.__enter__()
        g_ps = [ps.tile([128, 480], F32, tag="ps", name=f"g_ps{i}") for i in range(8)]
        for st in range(ST):
            et = embp.tile([128, C], F32R, tag="emb", name=f"et{st}")
            nc.sync.dma_start(out=et[:], in_=emb_d.ap()[st * 128 : (st + 1) * 128, :])
            at = eap.tile([128, KV], F32R, tag="ea", name=f"at{st}")
            nc.sync.dma_start(out=at[:], in_=ea_d.ap()[st * 128 : (st + 1) * 128, :])
            for ct in range(CT):
                for kc in range(2):
                    nc.tensor.matmul(
                        g_ps[ct * 2 + kc][:],
                        et[:, ct * 128 : (ct + 1) * 128],
                        at[:, kc * 480 : (kc + 1) * 480],
                        start=(st == 0),
                        stop=(st == ST - 1),
                    )
        for ct in range(CT):
            for kc in range(2):
                # Alternate ACT/DVE so the copy-out tail after the last G
                # matmul drains in half the time.
                dst = g_sb[:, ct, kc * 480 : (kc + 1) * 480]
                if (ct * 2 + kc) % 2 == 0:
                    nc.vector.tensor_copy(out=dst, in_=g_ps[ct * 2 + kc][:])
                else:
                    nc.scalar.copy(out=dst, in_=g_ps[ct * 2 + kc][:])
        gps_pool.__exit__(None, None, None)

        # ---- weights (host provides pre-transposed Wq.T / Wk.T / Wo.T) ------
        # Issued after the G-phase streams so the emb/emb_all DMAs (which
        # gate the first matmuls) get the HBM bandwidth first; within the
        # weights, in consumption order (wqt0 gates phase 2a).
        wqt_sb = []
        wkt_sb = []
        wv_sb = []
        for h in range(2):
            wq_t = wqp.tile([128, CT, C], F32R, tag="wq", name=f"wq{h}")
            nc.sync.dma_start(
                out=wq_t[:],
                in_=wqt_d.ap()[h].rearrange("(t p) d -> p t d", p=128),
            )
            wqt_sb.append(wq_t)
            wk_t = wkp.tile([128, KT, KV], F32R, tag="wk", name=f"wk{h}")
            wv_t = wvp.tile([128, KT, KV], F32R, tag="wv", name=f"wv{h}")
            for kt in range(KT):
                kp = _kp(kt)
                nc.sync.dma_start(
                    out=wk_t[:kp, kt, :],
                    in_=wkt_d.ap()[h, kt * 128 : kt * 128 + kp, :],
                )
            for kt in range(KT):
                kp = _kp(kt)
                nc.sync.dma_start(
                    out=wv_t[:kp, kt, :],
                    in_=wv_d.ap()[h, kt * 128 : kt * 128 + kp, :],
                )
            nc.vector.tensor_copy(out=wk_t[64:128, KT - 1, :], in_=zsrc[64:128, :])
            nc.vector.tensor_copy(out=wv_t[64:128, KT - 1, :], in_=zsrc[64:128, :])
            wkt_sb.append(wk_t)
            wv_sb.append(wv_t)
        wot_sb = wop.tile([128, CT, C], F32R)
        nc.sync.dma_start(
            out=wot_sb[:], in_=wot_d.ap().rearrange("(t p) d -> p t d", p=128)
        )

        # ---- phase 2: per-head scores -> instancenorm -> softmax -> Pv ------
        # The two heads are interleaved: h1's A matmuls are emitted between
        # h0's scoresT and h0's stats/softmax so the PE has work during the
        # (serial) stats chain. One shared PSUM pool spans phase 2 with tags
        # sized to exactly 8 banks: psa(2) + pw(4, shared by scoresT
        # accumulators and Pv accumulators) + one(2, shared by the two tiny
        # stats tiles and the softmax denominator).
        pbar_sb = pbp.tile([128, KT, C], F32R)
        ph2_pool = tc.tile_pool(name="ph2ps", bufs=1, space="PSUM")
        ps = ph2_pool.__enter__()
        hs = [{}, {}]

        def emit_A(h):
            d = hs[h]
            d["a_sb"] = a_sb = bigp.tile(
                [128, KT, C], F32R, tag="big", name=f"a_sb{h}"
            )
            for kt in range(KT):
                kp = _kp(kt)
                pa = ps.tile([128, C], F32, tag="psa", bufs=2, name=f"pa{h}{kt}")
                for ct in range(CT):
                    nc.tensor.matmul(
                        pa[:kp, :],
                        g_sb[:, ct, kt * 128 : kt * 128 + kp],
                        wqt_sb[h][:, ct, :],
                        start=(ct == 0),
                        stop=(ct == CT - 1),
                    )
                nc.vector.tensor_copy(out=a_sb[:kp, kt, :], in_=pa[:kp, :])
                if kt == KT - 1:
                    nc.vector.tensor_copy(
                        out=a_sb[64:128, kt, :], in_=zsrc[64:128, :C]
                    )

        def emit_scoresT(h):
            # scoresT[j, d] = sum_k WkT[k,j] A.T[k,d]; the reference's
            # 1/sqrt(KV) scale cancels through instance-norm (eps adjusted).
            # Per-jt stats partials run inline right behind each group.
            d = hs[h]
            a_sb = d["a_sb"]
            d["sc_sb"] = sc_sb = scp.tile(
                [128, KT, C], F32R, tag="sc", name=f"sc_sb{h}"
            )
            d["p_sb"] = p_sb = stp.tile([128, 16], F32, tag="p16", name=f"p_sb{h}")
            nc.vector.memset(p_sb[:], 0.0)
            prev_stop = None
            for jt in range(KT):
                jp = _kp(jt)
                pss = ps.tile([128, C], F32, tag="pw", bufs=4, name=f"pss{h}{jt}")
                for kt in range(KT):
                    mm = nc.tensor.matmul(
                        pss[:jp, :],
                        wkt_sb[h][:, kt, jt * 128 : jt * 128 + jp],
                        a_sb[:, kt, :],
                        start=(kt == 0),
                        stop=(kt == KT - 1),
                    )
                    # Keep the PE stream jt-group-major: otherwise the
                    # scheduler interleaves the groups and every stop lands
                    # at the tail, stalling the stats.
                    if kt == 0 and prev_stop is not None:
                        add_dep_helper(
                            mm.ins, prev_stop.ins, sync=False, reason="jt order"
                        )
                    if kt == KT - 1:
                        prev_stop = mm
                nc.scalar.copy(out=sc_sb[:jp, jt, :], in_=pss[:jp, :])
                nc.vector.reduce_sum(
                    out=p_sb[:jp, jt : jt + 1],
                    in_=pss[:jp, :],
                    axis=mybir.AxisListType.X,
                )
                nc.scalar.activation(
                    out=pss[:jp, :],
                    in_=pss[:jp, :],
                    func=mybir.ActivationFunctionType.Square,
                    accum_out=p_sb[:jp, 8 + jt : 9 + jt],
                )
            nc.vector.tensor_copy(out=sc_sb[64:128, KT - 1, :], in_=zsrc[64:128, :C])

        def emit_softmax_pv(h):
            d = hs[h]
            sc_sb = d["sc_sb"]
            p_sb = d["p_sb"]
            # cross-partition reduce + broadcast of the plane stats.
            p_r = stp.tile([128, 16], F32R, tag="p16r", name=f"p_r{h}")
            nc.vector.tensor_copy(out=p_r[:], in_=p_sb[:])
            pst = ps.tile([128, 16], F32, tag="one", bufs=2, name=f"pst{h}")
            nc.tensor.matmul(pst[:], ones[:], p_r[:], start=True, stop=True)
            n_inv = 1.0 / float(C * KV)
            sq2 = stp.tile([128, 2], F32, tag="sq2", name=f"sq2{h}")
            nc.vector.reduce_sum(
                out=sq2[:],
                in_=pst[:].rearrange("p (a b) -> p a b", a=2),
                axis=mybir.AxisListType.X,
            )
            # mean_neg = -sum/N; em2 = sumsq/N
            mean_neg = stp.tile([128, 1], F32, tag="mean", name=f"mean{h}")
            nc.vector.tensor_scalar(
                out=mean_neg[:], in0=sq2[:, 0:1], scalar1=-n_inv, scalar2=None,
                op0=mybir.AluOpType.mult,
            )
            em2 = stp.tile([128, 1], F32, tag="em2", name=f"em2{h}")
            nc.vector.tensor_scalar(
                out=em2[:], in0=sq2[:, 1:2], scalar1=n_inv, scalar2=None,
                op0=mybir.AluOpType.mult,
            )
            m2 = stp.tile([128, 1], F32, tag="m2", name=f"m2{h}")
            nc.vector.tensor_mul(out=m2[:], in0=mean_neg[:], in1=mean_neg[:])
            var_t = stp.tile([128, 1], F32, tag="var", name=f"var{h}")
            nc.vector.tensor_sub(out=var_t[:], in0=em2[:], in1=m2[:])
            std_t = stp.tile([128, 1], F32, tag="std", name=f"std{h}")
            nc.scalar.activation(
                out=std_t[:],
                in_=var_t[:],
                func=mybir.ActivationFunctionType.Sqrt,
                bias=eps_t[:],
            )
            # Swap the ACT table back to Exp while the DVE finishes the chain.
            prewarm(mybir.ActivationFunctionType.Exp, f"wex{h}")
            rstd_t = stp.tile([128, 1], F32, tag="rstd", name=f"rstd{h}")
            nc.vector.reciprocal(out=rstd_t[:], in_=std_t[:])
            negmr = stp.tile([128, 1], F32, tag="negmr", name=f"negmr{h}")
            nc.vector.tensor_mul(out=negmr[:], in0=mean_neg[:], in1=rstd_t[:])

            # softmax + Pv fused: Pv matmuls consume raw exp tiles as they
            # are produced; the 1/denom (per output column) and the 0.25
            # head-mean factor are applied on the copy-out.
            psd = ps.tile([128, C], F32, tag="one", bufs=2, name=f"psd{h}")
            pp_w1 = [
                ps.tile([128, C], F32, tag="pw", bufs=4, name=f"pp{h}w1_{kt}")
                for kt in range(4)
            ]
            for jt in range(KT):
                jp = _kp(jt)
                nc.scalar.activation(
                    out=sc_sb[:jp, jt, :],
                    in_=f(sc_sb[:jp, jt, :]),
                    func=mybir.ActivationFunctionType.Exp,
                    bias=negmr[:jp],
                    scale=rstd_t[:jp],
                )
                nc.tensor.matmul(
                    psd[:],
                    ones[:],
                    sc_sb[:, jt, :],
                    start=(jt == 0),
                    stop=(jt == KT - 1),
                )
                for kt in range(4):
                    nc.tensor.matmul(
                        pp_w1[kt][:, :],
                        wv_sb[h][:, jt, kt * 128 : (kt + 1) * 128],
                        sc_sb[:, jt, :],
                        start=(jt == 0),
                        stop=(jt == KT - 1),
                    )
            r4 = srp.tile([128, C], F32, tag="rd", name=f"r4{h}")
            nc.vector.reciprocal(out=r4[:], in_=psd[:])
            nc.scalar.mul(out=r4[:], in_=r4[:], mul=0.25)

            def pv_out(kt, pp):
                kp = _kp(kt)
                if h == 0:
                    nc.vector.tensor_mul(
                        out=pbar_sb[:kp, kt, :], in0=pp[:kp, :], in1=r4[:kp, :]
                    )
                else:
                    tmp = srp.tile([128, C], F32, tag="sr", name=f"tmp{kt}")
                    nc.vector.tensor_mul(
                        out=tmp[:kp, :], in0=pp[:kp, :], in1=r4[:kp, :]
                    )
                    nc.vector.tensor_add(
                        out=pbar_sb[:kp, kt, :],
                        in0=f(pbar_sb[:kp, kt, :]),
                        in1=tmp[:kp, :],
                    )

            # Wave 2a (kt 4,5) uses the psa banks, which are idle during the
            # softmax, so it streams right behind wave 1 without waiting for
            # wave 1's copy-outs; wave 2b (kt 6,7) reuses freed pw banks.
            pp_w2a = [
                ps.tile([128, C], F32, tag="psa", bufs=2, name=f"pp{h}w2a_{kt}")
                for kt in range(4, 6)
            ]
            for jt in range(KT):
                for kt in range(4, 6):
                    nc.tensor.matmul(
                        pp_w2a[kt - 4][:, :],
                        wv_sb[h][:, jt, kt * 128 : (kt + 1) * 128],
                        sc_sb[:, jt, :],
                        start=(jt == 0),
                        stop=(jt == KT - 1),
                    )
            # Wave-1 copy-outs run on the DVE while the PE streams wave 2.
            for kt in range(4):
                pv_out(kt, pp_w1[kt])
            pp_w2b = [
                ps.tile([128, C], F32, tag="pw", bufs=4, name=f"pp{h}w2b_{kt}")
                for kt in range(6, KT)
            ]
            for jt in range(KT):
                for kt in range(6, KT):
                    kp = _kp(kt)
                    nc.tensor.matmul(
                        pp_w2b[kt - 6][:kp, :],
                        wv_sb[h][:, jt, kt * 128 : kt * 128 + kp],
                        sc_sb[:, jt, :],
                        start=(jt == 0),
                        stop=(jt == KT - 1),
                    )
            for kt in range(4, 6):
                pv_out(kt, pp_w2a[kt - 4])
            for kt in range(6, KT):
                pv_out(kt, pp_w2b[kt - 6])

        emit_A(0)
        emit_scoresT(0)
        emit_A(1)
        emit_softmax_pv(0)
        emit_scoresT(1)
        emit_softmax_pv(1)
        ph2_pool.__exit__(None, None, None)

        # ---- phase 3: Z = Pbar @ Wo.T; y = emb_all @ Z ----------------------
        # Pbar.T via PE transposes ([c, k] layout).
        p3_pool = tc.tile_pool(name="p3ps", bufs=1, space="PSUM")
        ps = p3_pool.__enter__()
        pbt_sb = wkp.tile([128, CT, KV], F32R, tag="wk")
        for kt in range(KT):
            for ct in range(CT):
                kp = _kp(kt)
                ptr = ps.tile([128, 128], F32R, tag="ptc", bufs=3, name=f"ptr{ct}{kt}")
                nc.tensor.transpose(
                    ptr[:, :kp],
                    pbar_sb[:kp, kt, ct * 128 : (ct + 1) * 128],
                    ident[:kp, :kp],
                )
                nc.vector.tensor_copy(
                    out=pbt_sb[:, ct, kt * 128 : kt * 128 + kp], in_=f(ptr[:, :kp])
                )

        z_sb = bigp.tile([128, KT, C], F32R, tag="big")
        for kt in range(KT):
            kp = _kp(kt)
            pz = ps.tile([128, C], F32, tag="pz", bufs=2, name=f"pz{kt}")
            for ct in range(CT):
                nc.tensor.matmul(
                    pz[:kp, :],
                    pbt_sb[:, ct, kt * 128 : kt * 128 + kp],
                    wot_sb[:, ct, :],
                    start=(ct == 0),
                    stop=(ct == CT - 1),
                )
            nc.scalar.copy(out=z_sb[:kp, kt, :], in_=pz[:kp, :])
            if kt == KT - 1:
                nc.vector.tensor_copy(out=z_sb[64:128, kt, :], in_=zsrc[64:128, :C])

        # y rows: re-stream emb_all, transpose [s,k] chunks on the fly.
        # Transposes are emitted two chunks ahead of the consuming matmuls so
        # the PE never waits on the PSUM->SBUF copy between them.
        for st in range(ST):
            at2 = eap.tile([128, KV], F32R, tag="ea", name=f"at2_{st}")
            nc.sync.dma_start(out=at2[:], in_=ea_d.ap()[st * 128 : (st + 1) * 128, :])
            po = ps.tile([128, C], F32, tag="po", bufs=2, name=f"po{st}")
            trts = []

            def emit_tr(kt, at2=at2, trts=trts, st=st):
                kp = _kp(kt)
                ptc = ps.tile([128, 128], F32R, tag="ptc", bufs=3, name=f"ptc{st}{kt}")
                nc.tensor.transpose(
                    ptc[:kp, :], at2[:, kt * 128 : kt * 128 + kp], ident[:]
                )
                trt = trp.tile([128, 128], F32R, tag="tr", name=f"trt{st}{kt}")
                nc.vector.tensor_copy(out=trt[:kp, :], in_=f(ptc[:kp, :]))
                if kp < 128:
                    nc.vector.tensor_copy(
                        out=trt[64:128, :], in_=zsrc[64:128, :128]
                    )
                trts.append(trt)

            emit_tr(0)
            emit_tr(1)
            for kt in range(KT):
                if kt + 2 < KT:
                    emit_tr(kt + 2)
                nc.tensor.matmul(
                    po[:],
                    trts[kt][:, :],
                    z_sb[:, kt, :],
                    start=(kt == 0),
                    stop=(kt == KT - 1),
                )
            ot = outp.tile([128, C], F32, tag="out", name=f"ot{st}")
            nc.scalar.copy(out=ot[:], in_=po[:])
            nc.sync.dma_start(out=y_d.ap()[st * 128 : (st + 1) * 128, :], in_=ot[:])
        p3_pool.__exit__(None, None, None)

    nc.compile()
    return nc


_NC = None


def _get_nc():
    global _NC
    if _NC is None:
        _NC = _build_program()
    return _NC


def _in_maps(emb, emb_all, Wq, Wk, Wv, Wo):
    emb = np.ascontiguousarray(emb, dtype=np.float32)
    emb_all = np.ascontiguousarray(emb_all, dtype=np.float32)
    wot = np.ascontiguousarray(np.asarray(Wo, dtype=np.float32).T)
    maps = []
    for core in range(8):
        b, g = divmod(core, 2)
        h0 = 2 * g
        maps.append(
            {
                "emb": emb[b],
                "ea": emb_all[b],
                "wqt": np.ascontiguousarray(
                    np.asarray(Wq[h0 : h0 + 2], dtype=np.float32).transpose(0, 2, 1)
                ),
                "wkt": np.ascontiguousarray(
                    np.asarray(Wk[h0 : h0 + 2], dtype=np.float32).transpose(0, 2, 1)
                ),
                "wv": np.ascontiguousarray(np.asarray(Wv[h0 : h0 + 2], dtype=np.float32)),
                "wot": wot,
            }
        )
    return maps


def run(emb, emb_all, Wq, Wk, Wv, Wo, trace=False):
    nc = _get_nc()
    res = run_bass_kernel_spmd(
        nc, _in_maps(emb, emb_all, Wq, Wk, Wv, Wo), list(range(8)), trace=trace
    )
    out = np.empty((B, S, C), dtype=np.float32)
    for b in range(B):
        out[b] = res.results[2 * b]["y"] + res.results[2 * b + 1]["y"]
    return out, res


def kernel(emb, emb_all, Wq, Wk, Wv, Wo):
    out, _ = run(emb, emb_all, Wq, Wk, Wv, Wo, trace=False)
    return out



# revision 2
# speedup vs baseline: 1.2612x; 1.2612x over previous
"""Trainium2 Bass kernel for nn_Attention_1013612281902.

Reference computation (per batch b, head h):
    Q = emb @ Wq[h].T            [S,C]
    K = emb_all @ Wk[h].T        [S,KV]
    V = emb_all @ Wv[h].T        [S,KV]
    scores = Q.T @ K / sqrt(KV)  [C,KV]
    normed = instance_norm(scores)       (mean/var over the whole [C,KV] plane)
    probs  = softmax(normed, axis=KV)
    context = probs @ V.T        [C,S]
    out = mean_h(context).T @ Wo.T       [S,C]

Algebraic restructuring used here (S=4096 >> C=512, KV=960):
    G = emb.T @ emb_all                      [C,KV]   (shared across heads)
    scores = (Wq[h] @ G @ Wk[h].T)/sqrt(KV)
    Pv[h]  = probs[h] @ Wv[h]                [C,KV]
    out    = emb_all @ (mean_h Pv[h]).T @ Wo.T
This avoids materializing Q/K/V ([S,*] tensors) entirely and reduces the
FLOP count ~4x.

All matmul operands are bfloat16 (PSUM accumulation stays fp32): this
halves HBM traffic, enables fast-weight-load (FWL) so the per-matmul
LDWEIGHTS is hidden by the PE reorder window, and runs PE transposes at
1 cycle/row. Stats (mean/var) are computed from the f32 PSUM scores
before quantization; the tiny [128,16] cross-partition stats matmul
stays float32r so mean/var keep full precision. Measured end-to-end
rel err ~4e-3 vs the 2e-2 budget.

Sharding: 8 cores = (4 batches) x (2 head-pairs). Core 2b+g computes the
partial output for batch b over heads {2g, 2g+1}; the host adds the two
partials per batch (the head-mean and output projection are linear).
"""

import sys

if "/opt/trn_rl_repo" not in sys.path:
    sys.path.insert(0, "/opt/trn_rl_repo")

import math
from contextlib import ExitStack

import numpy as np
import ml_dtypes

import concourse.bacc as bacc
import concourse.mybir as mybir
import concourse.tile as tile
from concourse.bass_utils import run_bass_kernel_spmd
from concourse.masks import make_identity
from concourse.tile_rust import add_dep_helper

B, S, C, KV, H = 4, 4096, 512, 960, 4
EPS = 1e-5
F32 = mybir.dt.float32
F32R = mybir.dt.float32r
BF16 = mybir.dt.bfloat16

ST = S // 128           # 32 s-tiles
CT = C // 128           # 4 c-tiles
KT = (KV + 127) // 128  # 8 k-tiles (last one has 64 partitions)


def _kp(t):
    return min(128, KV - t * 128)


def _build_program():
    nc = bacc.Bacc("TRN2", target_bir_lowering=False, debug=False, num_devices=8)

    emb_d = nc.dram_tensor("emb", [S, C], BF16, kind="ExternalInput")
    ea_d = nc.dram_tensor("ea", [S, KV], BF16, kind="ExternalInput")
    wqt_d = nc.dram_tensor("wqt", [2, C, C], BF16, kind="ExternalInput")
    wkt_d = nc.dram_tensor("wkt", [2, KV, KV], BF16, kind="ExternalInput")
    wv_d = nc.dram_tensor("wv", [2, KV, KV], BF16, kind="ExternalInput")
    wot_d = nc.dram_tensor("wot", [C, C], BF16, kind="ExternalInput")
    y_d = nc.dram_tensor("y", [S, C], F32, kind="ExternalOutput")

    with tile.TileContext(nc) as tc, ExitStack() as ectx:
        ec = ectx.enter_context
        const = ec(tc.tile_pool(name="const", bufs=1))
        gp = ec(tc.tile_pool(name="gp", bufs=1))
        wqp = ec(tc.tile_pool(name="wqp", bufs=1))
        wkp = ec(tc.tile_pool(name="wkp", bufs=1))
        wvp = ec(tc.tile_pool(name="wvp", bufs=1))
        wop = ec(tc.tile_pool(name="wop", bufs=1))
        embp = ec(tc.tile_pool(name="embp", bufs=8))
        eap = ec(tc.tile_pool(name="eap", bufs=5))
        bigp = ec(tc.tile_pool(name="bigp", bufs=1))   # A then Z (sequential reuse)
        scp = ec(tc.tile_pool(name="scp", bufs=1))     # scoresT -> probsT
        pbp = ec(tc.tile_pool(name="pbp", bufs=1))     # Pbar accumulator
        trp = ec(tc.tile_pool(name="trp", bufs=8))
        outp = ec(tc.tile_pool(name="outp", bufs=4))
        srp = ec(tc.tile_pool(name="srp", bufs=2))     # [128,512] scratch
        stp = ec(tc.tile_pool(name="stp", bufs=4))     # small stats tiles

        ident = const.tile([128, 128], BF16)
        make_identity(nc, ident[:])
        ones = const.tile([128, 128], BF16)
        nc.vector.memset(ones[:], 1.0)
        # f32r ones + stats operand: the [128,16] cross-partition stats
        # matmul needs full fp32 precision (bf16 sums would feed var with
        # ~0.4% error straight into the softmax argument).
        onesf = const.tile([128, 128], F32)
        nc.vector.memset(onesf[:], 1.0)
        ones_r = const.tile([128, 128], F32R)
        nc.vector.tensor_copy(out=ones_r[:], in_=onesf[:])
        # scores are left unscaled (instance-norm is scale-invariant), so the
        # reference's eps applies to var/KV: use KV*eps against unscaled var.
        eps_t = const.tile([128, 1], F32)
        nc.vector.memset(eps_t[:], EPS * KV)
        # Scratch for ACT-table prewarming (Sqrt/Exp table loads are ~1.3us;
        # issuing a dummy op early moves the load off the critical chain).
        warm = const.tile([128, 1], F32)
        nc.vector.memset(warm[:], 1.0)

        def prewarm(func, nm):
            wsink = stp.tile([128, 1], F32, tag="wsink", name=nm)
            nc.scalar.activation(out=wsink[:], in_=warm[:], func=func)

        # ---- phase 1: G = emb.T @ emb_all  [C, KV] --------------------------
        g_sb = gp.tile([128, CT, KV], BF16)
        gps_pool = tc.tile_pool(name="gps", bufs=8, space="PSUM")
        ps = gps_pool.__enter__()
        g_ps = [ps.tile([128, 480], F32, tag="ps", name=f"g_ps{i}") for i in range(8)]
        for st in range(ST):
            et = embp.tile([128, C], BF16, tag="emb", name=f"et{st}")
            nc.sync.dma_start(out=et[:], in_=emb_d.ap()[st * 128 : (st + 1) * 128, :])
            at = eap.tile([128, KV], BF16, tag="ea", name=f"at{st}")
            nc.sync.dma_start(out=at[:], in_=ea_d.ap()[st * 128 : (st + 1) * 128, :])
            for ct in range(CT):
                for kc in range(2):
                    nc.tensor.matmul(
                        g_ps[ct * 2 + kc][:],
                        et[:, ct * 128 : (ct + 1) * 128],
                        at[:, kc * 480 : (kc + 1) * 480],
                        start=(st == 0),
                        stop=(st == ST - 1),
                    )
        for ct in range(CT):
            for kc in range(2):
                # Alternate ACT/DVE so the copy-out tail after the last G
                # matmul drains in half the time.
                dst = g_sb[:, ct, kc * 480 : (kc + 1) * 480]
                if (ct * 2 + kc) % 2 == 0:
                    nc.vector.tensor_copy(out=dst, in_=g_ps[ct * 2 + kc][:])
                else:
                    nc.scalar.copy(out=dst, in_=g_ps[ct * 2 + kc][:])
        gps_pool.__exit__(None, None, None)

        # ---- weights (host provides pre-transposed Wq.T / Wk.T / Wo.T) ------
        # Issued after the G-phase streams so the emb/emb_all DMAs (which
        # gate the first matmuls) get the HBM bandwidth first; within the
        # weights, in consumption order (wqt0 gates phase 2a).
        wqt_sb = []
        wkt_sb = []
        wv_sb = []
        for h in range(2):
            wq_t = wqp.tile([128, CT, C], BF16, tag="wq", name=f"wq{h}")
            nc.sync.dma_start(
                out=wq_t[:],
                in_=wqt_d.ap()[h].rearrange("(t p) d -> p t d", p=128),
            )
            wqt_sb.append(wq_t)
            wk_t = wkp.tile([128, KT, KV], BF16, tag="wk", name=f"wk{h}")
            wv_t = wvp.tile([128, KT, KV], BF16, tag="wv", name=f"wv{h}")
            for kt in range(KT):
                kp = _kp(kt)
                nc.sync.dma_start(
                    out=wk_t[:kp, kt, :],
                    in_=wkt_d.ap()[h, kt * 128 : kt * 128 + kp, :],
                )
            for kt in range(KT):
                kp = _kp(kt)
                nc.sync.dma_start(
                    out=wv_t[:kp, kt, :],
                    in_=wv_d.ap()[h, kt * 128 : kt * 128 + kp, :],
                )
            wkt_sb.append(wk_t)
            wv_sb.append(wv_t)
        wot_sb = wop.tile([128, CT, C], BF16)
        nc.sync.dma_start(
            out=wot_sb[:], in_=wot_d.ap().rearrange("(t p) d -> p t d", p=128)
        )

        # ---- phase 2: per-head scores -> instancenorm -> softmax -> Pv ------
        # The two heads are interleaved: h1's A matmuls are emitted between
        # h0's scoresT and h0's stats/softmax so the PE has work during the
        # (serial) stats chain. One shared PSUM pool spans phase 2 with tags
        # sized to exactly 8 banks: psa(2) + pw(4, shared by scoresT
        # accumulators and Pv accumulators) + one(2, shared by the two tiny
        # stats tiles and the softmax denominator).
        pbar_sb = pbp.tile([128, KT, C], BF16)
        ph2_pool = tc.tile_pool(name="ph2ps", bufs=1, space="PSUM")
        ps = ph2_pool.__enter__()
        hs = [{}, {}]

        def emit_A(h):
            d = hs[h]
            d["a_sb"] = a_sb = bigp.tile(
                [128, KT, C], BF16, tag="big", name=f"a_sb{h}"
            )
            for kt in range(KT):
                kp = _kp(kt)
                pa = ps.tile([128, C], F32, tag="psa", bufs=2, name=f"pa{h}{kt}")
                for ct in range(CT):
                    nc.tensor.matmul(
                        pa[:kp, :],
                        g_sb[:, ct, kt * 128 : kt * 128 + kp],
                        wqt_sb[h][:, ct, :],
                        start=(ct == 0),
                        stop=(ct == CT - 1),
                    )
                nc.vector.tensor_copy(out=a_sb[:kp, kt, :], in_=pa[:kp, :])

        def emit_scoresT(h):
            # scoresT[j, d] = sum_k WkT[k,j] A.T[k,d]; the reference's
            # 1/sqrt(KV) scale cancels through instance-norm (eps adjusted).
            # Per-jt stats partials run inline right behind each group.
            d = hs[h]
            a_sb = d["a_sb"]
            d["sc_sb"] = sc_sb = scp.tile(
                [128, KT, C], BF16, tag="sc", name=f"sc_sb{h}"
            )
            d["p_sb"] = p_sb = stp.tile([128, 16], F32, tag="p16", name=f"p_sb{h}")
            nc.vector.memset(p_sb[:], 0.0)
            prev_stop = None
            for jt in range(KT):
                jp = _kp(jt)
                pss = ps.tile([128, C], F32, tag="pw", bufs=4, name=f"pss{h}{jt}")
                for kt in range(KT):
                    kp = _kp(kt)
                    mm = nc.tensor.matmul(
                        pss[:jp, :],
                        wkt_sb[h][:kp, kt, jt * 128 : jt * 128 + jp],
                        a_sb[:kp, kt, :],
                        start=(kt == 0),
                        stop=(kt == KT - 1),
                    )
                    # Keep the PE stream jt-group-major: otherwise the
                    # scheduler interleaves the groups and every stop lands
                    # at the tail, stalling the stats.
                    if kt == 0 and prev_stop is not None:
                        add_dep_helper(
                            mm.ins, prev_stop.ins, sync=False, reason="jt order"
                        )
                    if kt == KT - 1:
                        prev_stop = mm
                nc.scalar.copy(out=sc_sb[:jp, jt, :], in_=pss[:jp, :])
                nc.vector.reduce_sum(
                    out=p_sb[:jp, jt : jt + 1],
                    in_=pss[:jp, :],
                    axis=mybir.AxisListType.X,
                )
                nc.scalar.activation(
                    out=pss[:jp, :],
                    in_=pss[:jp, :],
                    func=mybir.ActivationFunctionType.Square,
                    accum_out=p_sb[:jp, 8 + jt : 9 + jt],
                )

        def emit_softmax_pv(h):
            d = hs[h]
            sc_sb = d["sc_sb"]
            p_sb = d["p_sb"]
            # cross-partition reduce + broadcast of the plane stats.
            p_r = stp.tile([128, 16], F32R, tag="p16r", name=f"p_r{h}")
            nc.vector.tensor_copy(out=p_r[:], in_=p_sb[:])
            pst = ps.tile([128, 16], F32, tag="one", bufs=2, name=f"pst{h}")
            nc.tensor.matmul(pst[:], ones_r[:], p_r[:], start=True, stop=True)
            n_inv = 1.0 / float(C * KV)
            sq2 = stp.tile([128, 2], F32, tag="sq2", name=f"sq2{h}")
            nc.vector.reduce_sum(
                out=sq2[:],
                in_=pst[:].rearrange("p (a b) -> p a b", a=2),
                axis=mybir.AxisListType.X,
            )
            # mean_neg = -sum/N; em2 = sumsq/N
            mean_neg = stp.tile([128, 1], F32, tag="mean", name=f"mean{h}")
            nc.vector.tensor_scalar(
                out=mean_neg[:], in0=sq2[:, 0:1], scalar1=-n_inv, scalar2=None,
                op0=mybir.AluOpType.mult,
            )
            em2 = stp.tile([128, 1], F32, tag="em2", name=f"em2{h}")
            nc.vector.tensor_scalar(
                out=em2[:], in0=sq2[:, 1:2], scalar1=n_inv, scalar2=None,
                op0=mybir.AluOpType.mult,
            )
            m2 = stp.tile([128, 1], F32, tag="m2", name=f"m2{h}")
            nc.vector.tensor_mul(out=m2[:], in0=mean_neg[:], in1=mean_neg[:])
            var_t = stp.tile([128, 1], F32, tag="var", name=f"var{h}")
            nc.vector.tensor_sub(out=var_t[:], in0=em2[:], in1=m2[:])
            std_t = stp.tile([128, 1], F32, tag="std", name=f"std{h}")
            nc.scalar.activation(
                out=std_t[:],
                in_=var_t[:],
                func=mybir.ActivationFunctionType.Sqrt,
                bias=eps_t[:],
            )
            # Swap the ACT table back to Exp while the DVE finishes the chain.
            prewarm(mybir.ActivationFunctionType.Exp, f"wex{h}")
            rstd_t = stp.tile([128, 1], F32, tag="rstd", name=f"rstd{h}")
            nc.vector.reciprocal(out=rstd_t[:], in_=std_t[:])
            negmr = stp.tile([128, 1], F32, tag="negmr", name=f"negmr{h}")
            nc.vector.tensor_mul(out=negmr[:], in0=mean_neg[:], in1=rstd_t[:])

            # softmax + Pv fused: Pv matmuls consume raw exp tiles as they
            # are produced; the 1/denom (per output column) and the 0.25
            # head-mean factor are applied on the copy-out.
            psd = ps.tile([128, C], F32, tag="one", bufs=2, name=f"psd{h}")
            pp_w1 = [
                ps.tile([128, C], F32, tag="pw", bufs=4, name=f"pp{h}w1_{kt}")
                for kt in range(4)
            ]
            for jt in range(KT):
                jp = _kp(jt)
                nc.scalar.activation(
                    out=sc_sb[:jp, jt, :],
                    in_=sc_sb[:jp, jt, :],
                    func=mybir.ActivationFunctionType.Exp,
                    bias=negmr[:jp],
                    scale=rstd_t[:jp],
                )
                nc.tensor.matmul(
                    psd[:],
                    ones[:jp, :],
                    sc_sb[:jp, jt, :],
                    start=(jt == 0),
                    stop=(jt == KT - 1),
                )
                for kt in range(4):
                    nc.tensor.matmul(
                        pp_w1[kt][:, :],
                        wv_sb[h][:jp, jt, kt * 128 : (kt + 1) * 128],
                        sc_sb[:jp, jt, :],
                        start=(jt == 0),
                        stop=(jt == KT - 1),
                    )
            r4 = srp.tile([128, C], F32, tag="rd", name=f"r4{h}")
            nc.vector.reciprocal(out=r4[:], in_=psd[:])
            nc.scalar.mul(out=r4[:], in_=r4[:], mul=0.25)

            def pv_out(kt, pp):
                kp = _kp(kt)
                if h == 0:
                    nc.vector.tensor_mul(
                        out=pbar_sb[:kp, kt, :], in0=pp[:kp, :], in1=r4[:kp, :]
                    )
                else:
                    tmp = srp.tile([128, C], F32, tag="sr", name=f"tmp{kt}")
                    nc.vector.tensor_mul(
                        out=tmp[:kp, :], in0=pp[:kp, :], in1=r4[:kp, :]
                    )
                    nc.vector.tensor_add(
                        out=pbar_sb[:kp, kt, :],
                        in0=pbar_sb[:kp, kt, :],
                        in1=tmp[:kp, :],
                    )

            # Wave 2a (kt 4,5) uses the psa banks, which are idle during the
            # softmax, so it streams right behind wave 1 without waiting for
            # wave 1's copy-outs; wave 2b (kt 6,7) reuses freed pw banks.
            pp_w2a = [
                ps.tile([128, C], F32, tag="psa", bufs=2, name=f"pp{h}w2a_{kt}")
                for kt in range(4, 6)
            ]
            for jt in range(KT):
                jp = _kp(jt)
                for kt in range(4, 6):
                    nc.tensor.matmul(
                        pp_w2a[kt - 4][:, :],
                        wv_sb[h][:jp, jt, kt * 128 : (kt + 1) * 128],
                        sc_sb[:jp, jt, :],
                        start=(jt == 0),
                        stop=(jt == KT - 1),
                    )
            # Wave-1 copy-outs run on the DVE while the PE streams wave 2.
            for kt in range(4):
                pv_out(kt, pp_w1[kt])
            pp_w2b = [
                ps.tile([128, C], F32, tag="pw", bufs=4, name=f"pp{h}w2b_{kt}")
                for kt in range(6, KT)
            ]
            for jt in range(KT):
                jp = _kp(jt)
                for kt in range(6, KT):
                    kp = _kp(kt)
                    nc.tensor.matmul(
                        pp_w2b[kt - 6][:kp, :],
                        wv_sb[h][:jp, jt, kt * 128 : kt * 128 + kp],
                        sc_sb[:jp, jt, :],
                        start=(jt == 0),
                        stop=(jt == KT - 1),
                    )
            for kt in range(4, 6):
                pv_out(kt, pp_w2a[kt - 4])
            for kt in range(6, KT):
                pv_out(kt, pp_w2b[kt - 6])

        emit_A(0)
        emit_scoresT(0)
        emit_A(1)
        emit_softmax_pv(0)
        emit_scoresT(1)
        emit_softmax_pv(1)
        ph2_pool.__exit__(None, None, None)

        # ---- phase 3: Z = Pbar @ Wo.T; y = emb_all @ Z ----------------------
        # Pbar.T via PE transposes ([c, k] layout).
        p3_pool = tc.tile_pool(name="p3ps", bufs=1, space="PSUM")
        ps = p3_pool.__enter__()
        pbt_sb = wkp.tile([128, CT, KV], BF16, tag="wk")
        for kt in range(KT):
            for ct in range(CT):
                kp = _kp(kt)
                ptr = ps.tile(
                    [128, 128], BF16, tag="ptc", bufs=3, name=f"ptr{ct}{kt}"
                )
                nc.tensor.transpose(
                    ptr[:, :kp],
                    pbar_sb[:kp, kt, ct * 128 : (ct + 1) * 128],
                    ident[:kp, :kp],
                )
                nc.vector.tensor_copy(
                    out=pbt_sb[:, ct, kt * 128 : kt * 128 + kp], in_=ptr[:, :kp]
                )

        z_sb = bigp.tile([128, KT, C], BF16, tag="big")
        for kt in range(KT):
            kp = _kp(kt)
            pz = ps.tile([128, C], F32, tag="pz", bufs=2, name=f"pz{kt}")
            for ct in range(CT):
                nc.tensor.matmul(
                    pz[:kp, :],
                    pbt_sb[:, ct, kt * 128 : kt * 128 + kp],
                    wot_sb[:, ct, :],
                    start=(ct == 0),
                    stop=(ct == CT - 1),
                )
            nc.scalar.copy(out=z_sb[:kp, kt, :], in_=pz[:kp, :])

        # y rows: re-stream emb_all, transpose [s,k] chunks on the fly.
        # Transposes are emitted two chunks ahead of the consuming matmuls so
        # the PE never waits on the PSUM->SBUF copy between them.
        for st in range(ST):
            at2 = eap.tile([128, KV], BF16, tag="ea", name=f"at2_{st}")
            nc.sync.dma_start(out=at2[:], in_=ea_d.ap()[st * 128 : (st + 1) * 128, :])
            po = ps.tile([128, C], F32, tag="po", bufs=2, name=f"po{st}")
            trts = []

            def emit_tr(kt, at2=at2, trts=trts, st=st):
                kp = _kp(kt)
                ptc = ps.tile(
                    [128, 128], BF16, tag="ptc", bufs=3, name=f"ptc{st}{kt}"
                )
                nc.tensor.transpose(
                    ptc[:kp, :], at2[:, kt * 128 : kt * 128 + kp], ident[:]
                )
                trt = trp.tile([128, 128], BF16, tag="tr", name=f"trt{st}{kt}")
                nc.vector.tensor_copy(out=trt[:kp, :], in_=ptc[:kp, :])
                trts.append(trt)

            emit_tr(0)
            emit_tr(1)
            for kt in range(KT):
                kp = _kp(kt)
                if kt + 2 < KT:
                    emit_tr(kt + 2)
                nc.tensor.matmul(
                    po[:],
                    trts[kt][:kp, :],
                    z_sb[:kp, kt, :],
                    start=(kt == 0),
                    stop=(kt == KT - 1),
                )
            ot = outp.tile([128, C], F32, tag="out", name=f"ot{st}")
            nc.scalar.copy(out=ot[:], in_=po[:])
            nc.sync.dma_start(out=y_d.ap()[st * 128 : (st + 1) * 128, :], in_=ot[:])
        p3_pool.__exit__(None, None, None)

    nc.compile()
    return nc


_NC = None


def _get_nc():
    global _NC
    if _NC is None:
        _NC = _build_program()
    return _NC


def _in_maps(emb, emb_all, Wq, Wk, Wv, Wo):
    bfl = ml_dtypes.bfloat16
    emb = np.asarray(emb, dtype=np.float32).astype(bfl)
    emb_all = np.asarray(emb_all, dtype=np.float32).astype(bfl)
    wot = np.asarray(Wo, dtype=np.float32).T.astype(bfl)
    Wq = np.asarray(Wq, dtype=np.float32)
    Wk = np.asarray(Wk, dtype=np.float32)
    Wv = np.asarray(Wv, dtype=np.float32)
    maps = []
    for core in range(8):
        b, g = divmod(core, 2)
        h0 = 2 * g
        maps.append(
            {
                "emb": emb[b],
                "ea": emb_all[b],
                "wqt": Wq[h0 : h0 + 2].transpose(0, 2, 1).astype(bfl),
                "wkt": Wk[h0 : h0 + 2].transpose(0, 2, 1).astype(bfl),
                "wv": Wv[h0 : h0 + 2].astype(bfl),
                "wot": wot,
            }
        )
    return maps


def run(emb, emb_all, Wq, Wk, Wv, Wo, trace=False):
    nc = _get_nc()
    res = run_bass_kernel_spmd(
        nc, _in_maps(emb, emb_all, Wq, Wk, Wv, Wo), list(range(8)), trace=trace
    )
    out = np.empty((B, S, C), dtype=np.float32)
    for b in range(B):
        out[b] = res.results[2 * b]["y"] + res.results[2 * b + 1]["y"]
    return out, res


def kernel(emb, emb_all, Wq, Wk, Wv, Wo):
    out, _ = run(emb, emb_all, Wq, Wk, Wv, Wo, trace=False)
    return out
